# revision 6
# baseline (speedup 1.0000x reference)
"""DTM (distance-to-measure) kernel for Trainium2, 8 NeuronCores.

Math: for each (batch b, grid point g): with d2[m] = ||g - x_m||^2 and
bound = 0.3 * sum(w), the reference's sort+cumsum+searchsorted pipeline equals
  F(t*) = sum_{d2<t*} w*d2 + t* * (bound - sum_{d2<t*} w)
at the weighted-quantile threshold t*. F is continuous in t (jumps cancel), so
a dyadic bisection on t to precision eps gives |F(t)-F(t*)| <= eps * w_max —
no sort needed. Output = sqrt(F / (sum(w))) / sqrt(0.3)... (host does
sqrt(sel/wB)).

Device mapping (per core: one batch, 1664 grid points as 13 tiles of 128):
  PE:  d2 tile [128, 2048] via K=4 matmul  (rows [-2gx,-2gy,G2,1] x [x,y,1,X2])
  ACT: Relu copy PSUM->SBUF (clamps tiny negatives like the reference)
  DVE: per tile-pair interleaved bisection; each iteration is one fused
       scalar_tensor_tensor: mask=(d2<t), *w, accumulate -> weighted count,
       plus two tiny [128,1] tensor_scalar update ops.
Host: builds augmented matrices, shards, gathers, final sqrt (ScalarE sqrt
has a loose ULP budget; output is tiny so host sqrt is exact and free).
"""
import sys
sys.path.insert(0, "/opt/trn_rl_repo")

import numpy as np
import concourse.bass as bass
from concourse import mybir
from concourse.bass_utils import run_bass_kernel_spmd

M0 = 0.3
B, M, N = 2, 2048, 6561
P = 128
NT = 13              # tiles per core
NSH = NT * P         # 1664 grid points per core
NSHARDS = 4          # grid shards (x2 batches = 8 cores)
NPAD = NSH * NSHARDS # 6656
CHUNK = 512
NCH = M // CHUNK     # 4 moving chunks
ITERS = 12
REPS = 1             # bench amplifier: DVE program repeated REPS times
PHASE16 = True       # bisection compares on fp16 d2 (2x DVE mode); final pass f32
T0 = 4.0             # initial threshold midpoint; covers t* in (0, 8)

_NC = None


def _build():
    global _NC
    if _NC is not None:
        return _NC
    nc = bass.Bass()
    f32 = mybir.dt.float32

    gaug = nc.dram_tensor("gaug", [4, NSH], f32, kind="ExternalInput")
    xaug = nc.dram_tensor("xaug", [4, M], f32, kind="ExternalInput")
    f16 = mybir.dt.float16
    wrow = nc.dram_tensor("wrow", [1, M], f32, kind="ExternalInput")
    wrowh = nc.dram_tensor("wrowh", [1, M], f16, kind="ExternalInput")
    bnd = nc.dram_tensor("bnd", [1, 1], f32, kind="ExternalInput")
    bnd2 = nc.dram_tensor("bnd2", [1, 1], f32, kind="ExternalInput")
    t0in = nc.dram_tensor("t0in", [1, 1], f32, kind="ExternalInput")
    out = nc.dram_tensor("out", [P, NT], f32, kind="ExternalOutput")

    sb_gaug = nc.alloc_sbuf_tensor("sb_gaug", [4, NSH], f32)
    sb_xaug = nc.alloc_sbuf_tensor("sb_xaug", [4, M], f32)
    sb_w = nc.alloc_sbuf_tensor("sb_w", [P, M], f32)
    sb_wh = nc.alloc_sbuf_tensor("sb_wh", [P, M], f16)
    sb_bnd = nc.alloc_sbuf_tensor("sb_bnd", [P, 1], f32)
    sb_bnd2 = nc.alloc_sbuf_tensor("sb_bnd2", [P, 1], f32)
    sb_t0 = nc.alloc_sbuf_tensor("sb_t0", [P, 1], f32)
    sb_out = nc.alloc_sbuf_tensor("sb_out", [P, NT], f32)
    d2 = [nc.alloc_sbuf_tensor(f"d2_{t}", [P, M], f32) for t in range(NT)]
    d2h = [nc.alloc_sbuf_tensor(f"d2h_{t}", [P, M], f16) for t in range(NT)]
    scratch = nc.alloc_sbuf_tensor("scratch", [P, M], f32)
    scratchh = nc.alloc_sbuf_tensor("scratchh", [P, M], f16)
    # per-pair-slot state (slot 0/1 within an interleaved pair)
    tb = [[nc.alloc_sbuf_tensor(f"tb_{i}_{j}", [P, 1], f32) for j in range(2)]
          for i in range(2)]
    cnt = [nc.alloc_sbuf_tensor(f"cnt_{i}", [P, 1], f32) for i in range(2)]
    dirb = [nc.alloc_sbuf_tensor(f"dirb_{i}", [P, 1], f32) for i in range(2)]
    msum = [nc.alloc_sbuf_tensor(f"msum_{i}", [P, 1], f32) for i in range(2)]
    vv = [nc.alloc_sbuf_tensor(f"vv_{i}", [P, 1], f32) for i in range(2)]
    dmy = nc.alloc_sbuf_tensor("dmy", [P, 1], f32)
    ps = [nc.alloc_psum_tensor(f"ps_{i}", [P, M], f32) for i in range(2)]

    Alu = mybir.AluOpType
    Act = mybir.ActivationFunctionType

    with (
        nc.Block() as block,
        nc.semaphore("dma_sem") as dma_sem,
        nc.semaphore("mm_sem") as mm_sem,
        nc.semaphore("d2_sem") as d2_sem,
        nc.semaphore("dve_sem") as dve_sem,
    ):
        @block.sync
        def _(sync):
            sync.dma_start(out=sb_gaug[:], in_=gaug[:, :]).then_inc(dma_sem, 16)
            sync.dma_start(out=sb_xaug[:], in_=xaug[:, :]).then_inc(dma_sem, 16)
            sync.dma_start(out=sb_w[:], in_=wrow[:, :].to_broadcast((P, M))).then_inc(dma_sem, 16)
            sync.dma_start(out=sb_wh[:], in_=wrowh[:, :].to_broadcast((P, M))).then_inc(dma_sem, 16)
            sync.dma_start(out=sb_bnd[:], in_=bnd[:, :].to_broadcast((P, 1))).then_inc(dma_sem, 16)
            sync.dma_start(out=sb_bnd2[:], in_=bnd2[:, :].to_broadcast((P, 1))).then_inc(dma_sem, 16)
            sync.dma_start(out=sb_t0[:], in_=t0in[:, :].to_broadcast((P, 1))).then_inc(dma_sem, 16)

        @block.tensor
        def _(tensor):
            tensor.wait_ge(dma_sem, 32)
            for t in range(NT):
                if t >= 2:
                    tensor.wait_ge(d2_sem, t - 1)  # ACT drained ps[t%2]
                mm = None
                for c in range(NCH):
                    mm = tensor.matmul(
                        out=ps[t % 2][:, c * CHUNK:(c + 1) * CHUNK],
                        lhsT=sb_gaug[:, t * P:(t + 1) * P],
                        rhs=sb_xaug[:, c * CHUNK:(c + 1) * CHUNK],
                        start=True, stop=True)
                mm.then_inc(mm_sem, 1)

        @block.scalar
        def _(scalar):
            for t in range(NT):
                scalar.wait_ge(mm_sem, t + 1)
                if PHASE16:
                    scalar.activation(out=d2h[t][:], in_=ps[t % 2][:],
                                      func=Act.Relu)
                scalar.activation(out=d2[t][:], in_=ps[t % 2][:],
                                  func=Act.Relu).then_inc(d2_sem, 1)

        @block.vector
        def _(vector):
            vector.wait_ge(dma_sem, 112)
            last = None

            # pairs of tiles interleaved so every dependent read is >=2 ops old.
            # HW rule (measured): an op reading a value written by the
            # IMMEDIATELY preceding DVE op via the scalar/in1 port gets stale
            # data; rd0 (in0) 1-op-fresh reads are fine. The lone last tile
            # runs solo with dummy spacing ops where required.
            pairs = [(2 * i, 2 * i + 1) for i in range(NT // 2)] + [(NT - 1, None)]
            pairs = pairs * REPS
            for ta, tb_ in pairs:
                tiles = [ta] if tb_ is None else [ta, tb_]
                nsl = len(tiles)
                vector.wait_ge(d2_sem, max(tiles) + 1)

                def dummy():
                    # spacing no-op; reads long-settled sb_t0
                    vector.tensor_scalar(out=dmy[:], in0=sb_t0[:],
                                         scalar1=0.0, scalar2=None, op0=Alu.add)

                for i in range(nsl):
                    vector.tensor_scalar(out=tb[i][0][:], in0=sb_t0[:],
                                         scalar1=0.0, scalar2=None, op0=Alu.add)
                if nsl == 1:
                    dummy()
                s = T0 / 2.0
                for it in range(ITERS):
                    src_, dst = it % 2, (it + 1) % 2
                    for i in range(nsl):
                        if PHASE16:
                            vector.scalar_tensor_tensor(
                                out=scratchh[:], in0=d2h[tiles[i]][:],
                                scalar=tb[i][src_][:], in1=sb_wh[:],
                                op0=Alu.is_lt, op1=Alu.mult,
                                accum_out=cnt[i][:])
                        else:
                            vector.scalar_tensor_tensor(
                                out=scratch[:], in0=d2[tiles[i]][:],
                                scalar=tb[i][src_][:], in1=sb_w[:],
                                op0=Alu.is_lt, op1=Alu.mult,
                                accum_out=cnt[i][:])
                    for i in range(nsl):
                        vector.tensor_scalar(
                            out=dirb[i][:], in0=cnt[i][:], scalar1=sb_bnd[:],
                            scalar2=2.0 * s, op0=Alu.is_lt, op1=Alu.mult)
                    if nsl == 1:
                        dummy()
                    for i in range(nsl):
                        vector.tensor_scalar(
                            out=tb[i][dst][:], in0=tb[i][src_][:], scalar1=dirb[i][:],
                            scalar2=-s, op0=Alu.add, op1=Alu.add)
                    if nsl == 1:
                        dummy()
                    s *= 0.5
                tf = ITERS % 2
                # final: sel = sum(w*min(d2,t)) + t*(bound - sum(w))
                for i in range(nsl):
                    vector.scalar_tensor_tensor(
                        out=scratch[:], in0=d2[tiles[i]][:], scalar=tb[i][tf][:],
                        in1=sb_w[:], op0=Alu.min, op1=Alu.mult,
                        accum_out=msum[i][:])
                for i in range(nsl):
                    vector.tensor_scalar(out=vv[i][:], in0=tb[i][tf][:],
                                         scalar1=sb_bnd2[:], scalar2=None,
                                         op0=Alu.mult)
                for i in range(nsl):
                    last = vector.tensor_scalar(
                        out=sb_out[:, tiles[i]:tiles[i] + 1], in0=vv[i][:],
                        scalar1=msum[i][:], scalar2=None, op0=Alu.add)
            last.then_inc(dve_sem, 1)

        @block.sync
        def _(sync):
            sync.wait_ge(dve_sem, 1)
            sync.dma_start(out=out[:, :], in_=sb_out[:]).then_inc(dma_sem, 16)
            sync.wait_ge(dma_sem, 96)

    _NC = nc
    return nc


def _prepare_in_maps(inputs, weight, grid):
    inputs = np.asarray(inputs, dtype=np.float32)
    weight = np.asarray(weight, dtype=np.float32)
    grid = np.asarray(grid, dtype=np.float32)

    gpad = np.zeros((NPAD, 2), dtype=np.float32)
    gpad[:N] = grid
    G2 = (gpad * gpad).sum(-1)
    gaug_full = np.stack([-2.0 * gpad[:, 0], -2.0 * gpad[:, 1], G2,
                          np.ones(NPAD, np.float32)], 0).astype(np.float32)

    in_maps = []
    wB = np.empty(B, np.float32)
    for b in range(B):
        wB[b] = M0 * weight[b].sum(dtype=np.float32)
    t0 = np.array([[T0]], dtype=np.float32)
    for c in range(8):
        b = c // NSHARDS
        s = c % NSHARDS
        X = inputs[b]
        X2 = (X * X).sum(-1)
        xaug_np = np.stack([X[:, 0], X[:, 1], np.ones(M, np.float32), X2],
                           0).astype(np.float32)
        sw = weight[b].sum(dtype=np.float32)
        in_maps.append({
            "gaug": np.ascontiguousarray(gaug_full[:, s * NSH:(s + 1) * NSH]),
            "xaug": xaug_np,
            "wrow": weight[b:b + 1],
            "wrowh": weight[b:b + 1].astype(np.float16),
            "bnd": np.array([[wB[b]]], dtype=np.float32),
            "bnd2": np.array([[wB[b] - sw]], dtype=np.float32),
            "t0in": t0,
        })
    return in_maps, wB


def _gather(results, wB):
    sel = np.empty((B, NPAD), np.float32)
    for c in range(8):
        b = c // NSHARDS
        s = c % NSHARDS
        vals = results[c]["out"]            # [P, NT]; grid idx = t*P + p
        sel[b, s * NSH:(s + 1) * NSH] = vals.T.reshape(-1)
    sel = sel[:, :N]
    out = np.sqrt(np.maximum(sel, 0.0) / wB[:, None]).astype(np.float32)
    return out


def _make_runner(nc, n_cores=8):
    """Compile once; return a reusable sharded callable (avoids per-call
    retracing in run_bass_kernel_spmd)."""
    import jax
    from jax.sharding import Mesh, PartitionSpec
    from jax.experimental.shard_map import shard_map
    from concourse import bass2jax
    import concourse.mybir as _mybir

    bass2jax.install_neuronx_cc_hook()
    in_names, out_names, out_avals = [], [], []
    for alloc in nc.m.functions[0].allocations:
        if not isinstance(alloc, _mybir.MemoryLocationSet):
            continue
        name = alloc.memorylocations[0].name
        if alloc.kind == "ExternalInput":
            if not (nc.partition_id_tensor is not None
                    and name == nc.partition_id_tensor.name):
                in_names.append(name)
        elif alloc.kind == "ExternalOutput":
            out_names.append(name)
            out_avals.append(jax.core.ShapedArray(
                tuple(alloc.tensor_shape), _mybir.dt.np(alloc.dtype)))
    n_params = len(in_names)
    all_names = list(in_names) + list(out_names)
    has_pid = nc.partition_id_tensor is not None
    if has_pid:
        all_names.append(nc.partition_id_tensor.name)

    def _body(*args):
        operands = list(args)
        if has_pid:
            operands.append(bass2jax.partition_id_tensor())
        outs = bass2jax._bass_exec_p.bind(
            *operands, out_avals=tuple(out_avals), in_names=tuple(all_names),
            out_names=tuple(out_names), lowering_input_output_aliases=(),
            sim_require_finite=True, sim_require_nnan=True, nc=nc)
        return tuple(outs)

    devices = jax.devices()[:n_cores]
    mesh = Mesh(np.asarray(devices), ("core",))
    nio = n_params + len(out_names)
    sharded = jax.jit(
        shard_map(_body, mesh=mesh, in_specs=(PartitionSpec("core"),) * nio,
                  out_specs=(PartitionSpec("core"),) * len(out_names),
                  check_rep=False),
        keep_unused=True)

    def run(in_maps):
        import jax as _jax
        per_core = [[np.asarray(m[name]) for name in in_names] for m in in_maps]
        concat_in = [np.concatenate([per_core[c][i] for c in range(n_cores)], 0)
                     for i in range(n_params)]
        concat_zeros = [np.zeros((n_cores * a.shape[0], *a.shape[1:]), a.dtype)
                        for a in out_avals]
        outs = sharded(*concat_in, *concat_zeros)
        outs = [np.asarray(o) for o in outs]
        return [{name: outs[i].reshape(n_cores, *out_avals[i].shape)[c]
                 for i, name in enumerate(out_names)} for c in range(n_cores)]

    return run


_RUNNER = None


def _get_runner():
    global _RUNNER
    if _RUNNER is None:
        _RUNNER = _make_runner(_build())
    return _RUNNER


def kernel(inputs, weight, grid):
    run = _get_runner()
    in_maps, wB = _prepare_in_maps(inputs, weight, grid)
    results = run(in_maps)
    return _gather(results, wB)
